# revision 42
# baseline (speedup 1.0000x reference)
"""Multi-head attention TRN2 Bass kernel (8 NeuronCores, tensor-parallel).

Sharding: Megatron-style TP over (batch x head-group). 8 cores = 2 batches x 4
head-groups of 4 heads each. Each core computes its heads' Q/K/V projections,
masked-softmax attention, and a partial output projection; the host sums the 4
partials per batch (the TP unshard).

Final layout (pair-sweeps + row-tiled scores + JIT streaming):
  - Attention runs as 8 sweeps over (mq 512-wide m-chunk, hp head-pair). Per
    n-tile, the pair's two score matmuls are K=64 row-tiled (PE row groups 0
    and 64) and run CONCURRENTLY into the two banks of one sc tile; exp and
    the mask multiply treat the [A|B] tile as one 1024-wide op; ctx for the
    two heads lands in the column halves of one pctx tile (ones-row carries
    softmax sums).
  - Inputs stream column-major just-in-time: K-projection groups and
    V-projection rounds drip into sweep 0 between score pairs; Q-projection
    groups and the output projection drip into later sweeps.
  - DMA issue is spread across SP (kt/qt/out), ScalarE (wk/wq, ramp only)
    and GpSimd SWDGE (wv/wo/vt/keep) to beat the per-engine ~0.6us/issue
    serialization.
  - Normalize: one csb evacuation frees pctx fast; reciprocal_approx_fast on
    the sums row (partition 0), gpsimd partition_broadcast, two fused
    evacuate-multiplies. PSUM = sc[128,1024]f32 x3 + pctx x1 = 8 banks.
"""
import sys

for p in ("/opt/trn_rl_repo",):
    if p not in sys.path:
        sys.path.insert(0, p)

from contextlib import ExitStack

import numpy as np

import concourse.bass as bass
import concourse.tile as tile
from concourse import bacc, mybir
from concourse.bass_utils import run_bass_kernel_spmd

F32 = mybir.dt.float32
F16 = mybir.dt.float16
EXP = mybir.ActivationFunctionType.Exp

B, M, N, E = 2, 2048, 2048, 1024  # batch, q-len, k-len, d_model
H, DK = 16, 64                    # heads, head dim
NCORES = 8
GROUPS = 4                        # head groups (cores per batch)
DLOC = (H // GROUPS) * DK         # 256 per-core projection width
HL = H // GROUPS                  # 4 local heads
ET = E // 128                     # 8 e-tiles
NT = N // 128                     # 16 n-tiles
VSTR = HL * (DK + 1)              # 260: vw slot stride per n-tile
DEPTH = 4                         # ctx lag behind scores in the nt pipeline


def build_program() -> bass.Bass:
    nc = bacc.Bacc()

    qT_d = nc.dram_tensor("qT", [E, M], F16, kind="ExternalInput")
    kT_d = nc.dram_tensor("kT", [E, N], F16, kind="ExternalInput")
    vT_d = nc.dram_tensor("vT", [E, N], F16, kind="ExternalInput")
    keepT_d = nc.dram_tensor("keepT", [N, M], F16, kind="ExternalInput")
    wqT_d = nc.dram_tensor("wqT", [E, DLOC], F16, kind="ExternalInput")
    wkT_d = nc.dram_tensor("wkT", [E, DLOC], F16, kind="ExternalInput")
    wvT_d = nc.dram_tensor("wvT", [E, DLOC], F16, kind="ExternalInput")
    woT_d = nc.dram_tensor("woT", [DLOC, E], F16, kind="ExternalInput")
    out_d = nc.dram_tensor("out", [M, E], F16, kind="ExternalOutput")

    with tile.TileContext(nc) as tc, ExitStack() as ctx:
        persist = ctx.enter_context(tc.tile_pool(name="persist", bufs=1))
        ktp = ctx.enter_context(tc.tile_pool(name="ktp", bufs=24))
        vtp = ctx.enter_context(tc.tile_pool(name="vtp", bufs=16))
        qtp = ctx.enter_context(tc.tile_pool(name="qtp", bufs=16))
        ps_pool = ctx.enter_context(tc.tile_pool(name="ps", bufs=1, space="PSUM"))
        nrm_pool = ctx.enter_context(tc.tile_pool(name="nrm", bufs=1))
        au_pool = ctx.enter_context(tc.tile_pool(name="au", bufs=4))
        am_pool = ctx.enter_context(tc.tile_pool(name="am", bufs=5))
        ob_pool = ctx.enter_context(tc.tile_pool(name="ob", bufs=4))

        warm = persist.tile([1, 64], F16, tag="warm")
        wq_sb = persist.tile([128, ET * DLOC], F16, tag="wq")
        wk_sb = persist.tile([128, ET * DLOC], F16, tag="wk")
        wv_sb = persist.tile([128, ET * DLOC], F16, tag="wv")
        wo_sb = persist.tile([128, 2 * E], F16, tag="wo")

        # qw/kw: [d-rows, seq]; pair hp = heads {2hp, 2hp+1} = top/bottom
        # 64-partition halves — exactly the row-tiling split.
        qw_sb = [persist.tile([128, M], F16, tag=f"qw{i}", name=f"qw{i}") for i in range(2)]
        kw_sb = [persist.tile([128, N], F16, tag=f"kw{i}", name=f"kw{i}") for i in range(2)]
        vw_sb = persist.tile([128, NT * VSTR], F16, tag="vw")
        nc.vector.memset(vw_sb[:], 1.0)  # ones cols persist; data overwritten
        ctx_sb = [persist.tile([128, M], F16, tag=f"ctx{i}", name=f"ctx{i}") for i in range(2)]
        keep_sb = persist.tile([128, NT * M], F16, tag="keep")
        sum_sb = persist.tile([1, 1024], F32, tag="sum")
        rbs_sb = persist.tile([64, 1024], F32, tag="rbs")

        # ---- DMA issue: ACT: wk+wq; GpSimd: wv+wo+vt+keep; SP: kt+qt ----
        for et in range(ET):
            nc.scalar.dma_start(wk_sb[:, bass.ts(et, DLOC)], wkT_d[bass.ts(et, 128), :])
        for et in range(ET):
            nc.scalar.dma_start(wq_sb[:, bass.ts(et, DLOC)], wqT_d[bass.ts(et, 128), :])
        for et in range(ET):
            nc.gpsimd.dma_start(wv_sb[:, bass.ts(et, DLOC)], wvT_d[bass.ts(et, 128), :])
        for kt2 in range(2):
            nc.gpsimd.dma_start(wo_sb[:, bass.ts(kt2, E)], woT_d[bass.ts(kt2, 128), :])

        # column chunks: kc[n4][et], qc[m4][et] on SP; vc[q][et] + keep on
        # gpsimd. Issue order is need order: kc0+qc0 first, keep interleaved
        # with vc so the first masks aren't starved behind 32 vc issues.
        kc = [[None] * ET for _ in range(4)]
        qc = [[None] * ET for _ in range(4)]
        vc = [[None] * ET for _ in range(4)]

        def kc_dma(n4, split=False):
            for et in range(ET):
                t = ktp.tile([128, 512], F16, tag="kc", name=f"kc{n4}_{et}")
                if split:
                    nc.sync.dma_start(
                        t[:, 0:256], kT_d[bass.ts(et, 128), n4 * 512 : n4 * 512 + 256]
                    )
                else:
                    nc.sync.dma_start(t[:], kT_d[bass.ts(et, 128), bass.ts(n4, 512)])
                kc[n4][et] = t

        def qc_dma(m4, split=False):
            for et in range(ET):
                t = qtp.tile([128, 512], F16, tag="qc", name=f"qc{m4}_{et}")
                if split:
                    nc.sync.dma_start(
                        t[:, 0:256], qT_d[bass.ts(et, 128), m4 * 512 : m4 * 512 + 256]
                    )
                else:
                    nc.sync.dma_start(t[:], qT_d[bass.ts(et, 128), bass.ts(m4, 512)])
                qc[m4][et] = t

        def vc_dma(q):
            for et in range(ET):
                t = vtp.tile([128, 512], F16, tag="vc", name=f"vc{q}_{et}")
                nc.gpsimd.dma_start(t[:], vT_d[bass.ts(et, 128), bass.ts(q, 512)])
                vc[q][et] = t

        def keep_dma(lo, hi):
            for nt in range(lo, hi):
                nc.sync.dma_start(
                    keep_sb[:, bass.ts(nt, M)], keepT_d[bass.ts(nt, 128), :]
                )

        kc_dma(0)
        qc_dma(0)
        keep_dma(0, 2)
        kc_dma(1)
        keep_dma(2, 4)
        kc_dma(2)
        keep_dma(4, 6)
        kc_dma(3)
        keep_dma(6, 9)
        qc_dma(1)
        keep_dma(9, 12)
        qc_dma(2)
        keep_dma(12, 16)
        qc_dma(3)
        vc_dma(0)
        vc_dma(1)
        vc_dma(2)
        vc_dma(3)

        # exp table warm-up (after DMA issue so it doesn't delay ScalarE's
        # weight dma_starts at t=0; the table load only must beat sweep 0)
        nc.vector.memset(warm[:], 1.0)
        nc.scalar.activation(warm[:], warm[:], EXP, scale=0.125)

        # ---- projection group emitters (dripped between score pairs) ----
        def k_group(d2, n4):
            ps = ps_pool.tile([128, 1024], F32, tag="sc", bufs=3, name=f"kp{d2}{n4}")
            for et in range(ET):
                nc.tensor.matmul(
                    ps[:, 0:512],
                    wk_sb[:, et * DLOC + d2 * 128 : et * DLOC + (d2 + 1) * 128],
                    kc[n4][et][:],
                    start=(et == 0), stop=(et == ET - 1),
                )
            nc.scalar.copy(kw_sb[d2][:, bass.ts(n4, 512)], ps[:, 0:512])

        def q_group(d2, m4):
            ps = ps_pool.tile([128, 1024], F32, tag="sc", bufs=3, name=f"qp{d2}{m4}")
            for et in range(ET):
                nc.tensor.matmul(
                    ps[:, 0:512],
                    wq_sb[:, et * DLOC + d2 * 128 : et * DLOC + (d2 + 1) * 128],
                    qc[m4][et][:],
                    start=(et == 0), stop=(et == ET - 1),
                )
            nc.vector.tensor_copy(qw_sb[d2][:, bass.ts(m4, 512)], ps[:, 0:512])

        def v_round(nt):
            q, r = divmod(nt, 4)
            ps = ps_pool.tile([128, 1024], F32, tag="sc", bufs=3, name=f"vp{nt}")
            for et in range(ET):
                nc.tensor.matmul(
                    ps[:, 0:DLOC],
                    vc[q][et][:, bass.ts(r, 128)],
                    wv_sb[:, bass.ts(et, DLOC)],
                    start=(et == 0), stop=(et == ET - 1),
                )
            src = ps[:, 0:DLOC].rearrange("p (h c) -> p h c", h=HL)
            dst = vw_sb[:, nt * VSTR : (nt + 1) * VSTR].rearrange(
                "p (h c) -> p h c", h=HL
            )[:, :, 0:DK]
            nc.scalar.copy(dst, src)

        def out_group(mt, split_copy=False):
            po = ps_pool.tile([128, 1024], F32, tag="sc", bufs=3, name=f"po{mt}")
            for kt2 in range(2):
                for ec in range(2):
                    nc.tensor.matmul(
                        po[:, bass.ts(ec, 512)],
                        ctx_sb[kt2][:, bass.ts(mt, 128)],
                        wo_sb[:, kt2 * E + ec * 512 : kt2 * E + (ec + 1) * 512],
                        start=(kt2 == 0), stop=(kt2 == 1),
                    )
            for ec in range(2):
                ob = ob_pool.tile([128, 512], F16, tag="ob", name=f"ob{mt}{ec}")
                if split_copy and ec == 1:
                    nc.scalar.copy(ob[:], po[:, bass.ts(ec, 512)])
                else:
                    nc.vector.tensor_copy(ob[:], po[:, bass.ts(ec, 512)])
                if split_copy:
                    nc.sync.dma_start(
                        out_d[bass.ts(mt, 128), ec * 512 : ec * 512 + 256],
                        ob[:, 0:256],
                    )
                    nc.sync.dma_start(
                        out_d[bass.ts(mt, 128), ec * 512 + 256 : (ec + 1) * 512],
                        ob[:, 256:512],
                    )
                else:
                    nc.sync.dma_start(
                        out_d[bass.ts(mt, 128), bass.ts(ec, 512)], ob[:]
                    )

        # drip schedule: sweep s = 2*mq + hp; drips[s][nt] = emitter
        def D(fn, *a):
            return lambda: fn(*a)

        drips = {s: {} for s in range(8)}
        drips[0] = {
            1: D(k_group, 0, 1), 2: D(k_group, 1, 0), 3: D(k_group, 0, 2),
            4: D(k_group, 1, 1), 5: D(k_group, 0, 3), 6: D(k_group, 1, 2),
            7: D(k_group, 1, 3), 9: D(q_group, 1, 0),
        }
        drips[1] = {3: D(q_group, 0, 1), 9: D(q_group, 1, 1)}
        drips[2] = {3: D(q_group, 0, 2), 9: D(q_group, 1, 2),
                    7: D(out_group, 0)}
        drips[3] = {3: D(q_group, 0, 3), 9: D(q_group, 1, 3),
                    7: D(out_group, 2)}
        drips[4] = {7: D(out_group, 4)}
        drips[5] = {7: D(out_group, 6)}
        drips[6] = {7: D(out_group, 8)}
        drips[7] = {7: D(out_group, 10)}

        # K(0,0) and Q(0,0) must precede sweep 0; V rounds drip per-nt inside.
        k_group(0, 0)
        q_group(0, 0)

        # ---- attention pair-sweeps ----
        # norm for sweep s runs piecewise inside sweep s+1 so the exp/mask
        # streams never queue behind the whole chain at a boundary.
        def norm_piece(state, piece):
            hp0, moff0, pctx0, csb0 = state
            if piece == 0:
                nc.vector.tensor_copy(csb0[0:65, :], pctx0[0:65, :])
            elif piece == 1:
                nc.vector.tensor_copy(sum_sb[:], csb0[64:65, :])
                nc.vector.reciprocal_approx_fast(sum_sb[:], sum_sb[:])
                nc.gpsimd.partition_broadcast(rbs_sb[:], sum_sb[0:1, :])
            else:
                i = piece - 2
                nc.vector.tensor_mul(
                    ctx_sb[hp0][bass.ts(i, 64), moff0 : moff0 + 512],
                    csb0[0:64, bass.ts(i, 512)],
                    rbs_sb[0:64, bass.ts(i, 512)],
                )

        pending = None
        for s in range(8):
            mq, hp = divmod(s, 2)
            moff = mq * 512
            pctx = ps_pool.tile([128, 1024], F32, tag="pctx", bufs=1, name=f"pctx{s}")
            ams = {}
            for step in range(NT + DEPTH):
                if step < NT and pending is not None and 1 <= step <= 4:
                    norm_piece(pending, step - 1)
                    if step == 4:
                        pending = None
                if step < NT:
                    nt = step
                    sc = ps_pool.tile(
                        [128, 1024], F32, tag="sc", bufs=3, name=f"sc{s}_{nt}"
                    )
                    # row-tiled concurrent pair: head A rows 0:64, B rows 64:128
                    nc.tensor.matmul(
                        sc[:, 0:512],
                        kw_sb[hp][0:64, bass.ts(nt, 128)],
                        qw_sb[hp][0:64, moff : moff + 512],
                        start=True, stop=True,
                    )
                    nc.tensor.matmul(
                        sc[:, 512:1024],
                        kw_sb[hp][64:128, bass.ts(nt, 128)],
                        qw_sb[hp][64:128, moff : moff + 512],
                        start=True, stop=True,
                    )
                    if s == 0 and nt >= 2:
                        v_round(nt - 2)
                    if s == 0 and nt == 15:
                        v_round(14)
                        v_round(15)
                    au = au_pool.tile([128, 1024], F16, tag="au", name=f"au{s}_{nt}")
                    nc.scalar.activation(au[:], sc[:], EXP, scale=0.125)
                    am = am_pool.tile([128, 1024], F16, tag="am", name=f"am{s}_{nt}")
                    ka, kb = bass.broadcast_tensor_aps(
                        au[:].rearrange("p (o c) -> p o c", o=2),
                        keep_sb[:, nt * M + moff : nt * M + moff + 512].rearrange(
                            "p (o c) -> p o c", o=1
                        ),
                    )
                    nc.vector.tensor_mul(
                        am[:].rearrange("p (o c) -> p o c", o=2), ka, kb
                    )
                    ams[nt] = am
                    fn = drips[s].get(nt)
                    if fn:
                        fn()
                if step >= DEPTH:
                    nt = step - DEPTH
                    am = ams.pop(nt)
                    for i in range(2):
                        h = 2 * hp + i
                        nc.tensor.matmul(
                            pctx[0:65, bass.ts(i, 512)],
                            vw_sb[:, nt * VSTR + h * 65 : nt * VSTR + (h + 1) * 65],
                            am[:, bass.ts(i, 512)],
                            start=(nt == 0), stop=(nt == NT - 1),
                        )
            # stash this sweep's normalize; it runs inside the next sweep
            csb = nrm_pool.tile([65, 1024], F32, tag="csb", name=f"csb{s}")
            pending = (hp, moff, pctx, csb)

        # final sweep's norm runs immediately, then the tail
        for piece in range(4):
            norm_piece(pending, piece)

        # ---- output projection tail (odd m-tiles + last m-chunk) ----
        for mt in (1, 3, 5, 7, 9, 11, 12, 13, 14, 15):
            out_group(mt, split_copy=True)

    nc.finalize()
    return nc


_PROGRAM = None


def _get_program():
    global _PROGRAM
    if _PROGRAM is None:
        _PROGRAM = build_program()
    return _PROGRAM


def _make_in_maps(q, k, v, mask, Wq, Wk, Wv, Wo):
    q = np.asarray(q, dtype=np.float32)
    k = np.asarray(k, dtype=np.float32)
    v = np.asarray(v, dtype=np.float32)
    mask = np.asarray(mask)
    Wq = np.asarray(Wq, dtype=np.float32)
    Wk = np.asarray(Wk, dtype=np.float32)
    Wv = np.asarray(Wv, dtype=np.float32)
    Wo = np.asarray(Wo, dtype=np.float32)

    per_batch = {}
    for b in range(B):
        per_batch[b] = dict(
            qT=np.ascontiguousarray(q[b].T.astype(np.float16)),
            kT=np.ascontiguousarray(k[b].T.astype(np.float16)),
            vT=np.ascontiguousarray(v[b].T.astype(np.float16)),
            keepT=np.ascontiguousarray(
                np.logical_not(mask[b]).T.astype(np.float16)
            ),
        )

    in_maps = []
    for c in range(NCORES):
        b, hg = divmod(c, GROUPS)
        sl = slice(hg * DLOC, (hg + 1) * DLOC)
        in_maps.append(
            dict(
                per_batch[b],
                wqT=np.ascontiguousarray(Wq[sl].T.astype(np.float16)),
                wkT=np.ascontiguousarray(Wk[sl].T.astype(np.float16)),
                wvT=np.ascontiguousarray(Wv[sl].T.astype(np.float16)),
                woT=np.ascontiguousarray(Wo[:, sl].T.astype(np.float16)),
            )
        )
    return in_maps


def _run(in_maps, trace=False):
    nc = _get_program()
    return run_bass_kernel_spmd(
        nc, in_maps, list(range(NCORES)), trace=trace
    )


def _assemble(results):
    out = np.zeros((B, M, E), dtype=np.float32)
    for c in range(NCORES):
        b = c // GROUPS
        out[b] += results[c]["out"].astype(np.float32)
    return out


def kernel(q, k, v, mask, Wq, Wk, Wv, Wo):
    in_maps = _make_in_maps(q, k, v, mask, Wq, Wk, Wv, Wo)
    res = _run(in_maps, trace=False)
    return _assemble(res.results)


def run_profiled(q, k, v, mask, Wq, Wk, Wv, Wo):
    """Like kernel(), but traces execution; returns (out, BassKernelResults)."""
    in_maps = _make_in_maps(q, k, v, mask, Wq, Wk, Wv, Wo)
    res = _run(in_maps, trace=True)
    return _assemble(res.results), res


# revision 44
# speedup vs baseline: 1.0323x; 1.0323x over previous
"""Multi-head attention TRN2 Bass kernel (8 NeuronCores, tensor-parallel).

Sharding: Megatron-style TP over (batch x head-group). 8 cores = 2 batches x 4
head-groups of 4 heads each. Each core computes its heads' Q/K/V projections,
masked-softmax attention, and a partial output projection; the host sums the 4
partials per batch (the TP unshard).

Final layout (pair-sweeps + row-tiled scores + JIT streaming):
  - Attention runs as 8 sweeps over (mq 512-wide m-chunk, hp head-pair). Per
    n-tile, the pair's two score matmuls are K=64 row-tiled (PE row groups 0
    and 64) and run CONCURRENTLY into the two banks of one sc tile; exp and
    the mask multiply treat the [A|B] tile as one 1024-wide op; ctx for the
    two heads lands in the column halves of one pctx tile (ones-row carries
    softmax sums).
  - Inputs stream column-major just-in-time: K-projection groups and
    V-projection rounds drip into sweep 0 between score pairs; Q-projection
    groups and the output projection drip into later sweeps.
  - DMA issue is spread across SP (kt/qt/out), ScalarE (wk/wq, ramp only)
    and GpSimd SWDGE (wv/wo/vt/keep) to beat the per-engine ~0.6us/issue
    serialization.
  - Normalize: one csb evacuation frees pctx fast; reciprocal_approx_fast on
    the sums row (partition 0), gpsimd partition_broadcast, two fused
    evacuate-multiplies. PSUM = sc[128,1024]f32 x3 + pctx x1 = 8 banks.
"""
import sys

for p in ("/opt/trn_rl_repo",):
    if p not in sys.path:
        sys.path.insert(0, p)

from contextlib import ExitStack

import numpy as np

import concourse.bass as bass
import concourse.tile as tile
from concourse import bacc, mybir
from concourse.bass_utils import run_bass_kernel_spmd

F32 = mybir.dt.float32
F16 = mybir.dt.float16
EXP = mybir.ActivationFunctionType.Exp

B, M, N, E = 2, 2048, 2048, 1024  # batch, q-len, k-len, d_model
H, DK = 16, 64                    # heads, head dim
NCORES = 8
GROUPS = 4                        # head groups (cores per batch)
DLOC = (H // GROUPS) * DK         # 256 per-core projection width
HL = H // GROUPS                  # 4 local heads
ET = E // 128                     # 8 e-tiles
NT = N // 128                     # 16 n-tiles
VSTR = HL * (DK + 1)              # 260: vw slot stride per n-tile
DEPTH = 4                         # ctx lag behind scores in the nt pipeline


def build_program() -> bass.Bass:
    nc = bacc.Bacc()

    qT_d = nc.dram_tensor("qT", [E, M], F16, kind="ExternalInput")
    kT_d = nc.dram_tensor("kT", [E, N], F16, kind="ExternalInput")
    vT_d = nc.dram_tensor("vT", [E, N], F16, kind="ExternalInput")
    keepT_d = nc.dram_tensor("keepT", [N, M], F16, kind="ExternalInput")
    wqT_d = nc.dram_tensor("wqT", [E, DLOC], F16, kind="ExternalInput")
    wkT_d = nc.dram_tensor("wkT", [E, DLOC], F16, kind="ExternalInput")
    wvT_d = nc.dram_tensor("wvT", [E, DLOC], F16, kind="ExternalInput")
    woT_d = nc.dram_tensor("woT", [DLOC, E], F16, kind="ExternalInput")
    out_d = nc.dram_tensor("out", [M, E], F16, kind="ExternalOutput")

    with tile.TileContext(nc) as tc, ExitStack() as ctx:
        persist = ctx.enter_context(tc.tile_pool(name="persist", bufs=1))
        ktp = ctx.enter_context(tc.tile_pool(name="ktp", bufs=24))
        vtp = ctx.enter_context(tc.tile_pool(name="vtp", bufs=16))
        qtp = ctx.enter_context(tc.tile_pool(name="qtp", bufs=16))
        ps_pool = ctx.enter_context(tc.tile_pool(name="ps", bufs=1, space="PSUM"))
        nrm_pool = ctx.enter_context(tc.tile_pool(name="nrm", bufs=1))
        au_pool = ctx.enter_context(tc.tile_pool(name="au", bufs=5))
        am_pool = ctx.enter_context(tc.tile_pool(name="am", bufs=5))
        ob_pool = ctx.enter_context(tc.tile_pool(name="ob", bufs=2))

        warm = persist.tile([1, 64], F16, tag="warm")
        wq_sb = persist.tile([128, ET * DLOC], F16, tag="wq")
        wk_sb = persist.tile([128, ET * DLOC], F16, tag="wk")
        wv_sb = persist.tile([128, ET * DLOC], F16, tag="wv")
        wo_sb = persist.tile([128, 2 * E], F16, tag="wo")

        # qw/kw: [d-rows, seq]; pair hp = heads {2hp, 2hp+1} = top/bottom
        # 64-partition halves — exactly the row-tiling split.
        qw_sb = [persist.tile([128, M], F16, tag=f"qw{i}", name=f"qw{i}") for i in range(2)]
        kw_sb = [persist.tile([128, N], F16, tag=f"kw{i}", name=f"kw{i}") for i in range(2)]
        vw_sb = persist.tile([128, NT * VSTR], F16, tag="vw")
        nc.vector.memset(vw_sb[:], 1.0)  # ones cols persist; data overwritten
        ctx_sb = [persist.tile([128, M], F16, tag=f"ctx{i}", name=f"ctx{i}") for i in range(2)]
        keep_sb = persist.tile([128, NT * M], F16, tag="keep")
        sum_sb = persist.tile([1, 1024], F32, tag="sum")
        rbs_sb = persist.tile([64, 1024], F32, tag="rbs")

        # ---- DMA issue: ACT: wk+wq; GpSimd: wv+wo+vt+keep; SP: kt+qt ----
        for et in range(ET):
            nc.scalar.dma_start(wk_sb[:, bass.ts(et, DLOC)], wkT_d[bass.ts(et, 128), :])
        for et in range(ET):
            nc.scalar.dma_start(wq_sb[:, bass.ts(et, DLOC)], wqT_d[bass.ts(et, 128), :])
        for et in range(ET):
            nc.gpsimd.dma_start(wv_sb[:, bass.ts(et, DLOC)], wvT_d[bass.ts(et, 128), :])
        for kt2 in range(2):
            nc.gpsimd.dma_start(wo_sb[:, bass.ts(kt2, E)], woT_d[bass.ts(kt2, 128), :])

        # column chunks: kc[n4][et], qc[m4][et] on SP; vc[q][et] + keep on
        # gpsimd. Issue order is need order: kc0+qc0 first, keep interleaved
        # with vc so the first masks aren't starved behind 32 vc issues.
        kc = [[None] * ET for _ in range(4)]
        qc = [[None] * ET for _ in range(4)]
        vc = [[None] * ET for _ in range(4)]

        def kc_dma(n4, split=False):
            for et in range(ET):
                t = ktp.tile([128, 512], F16, tag="kc", name=f"kc{n4}_{et}")
                if split:
                    nc.sync.dma_start(
                        t[:, 0:256], kT_d[bass.ts(et, 128), n4 * 512 : n4 * 512 + 256]
                    )
                else:
                    nc.sync.dma_start(t[:], kT_d[bass.ts(et, 128), bass.ts(n4, 512)])
                kc[n4][et] = t

        def qc_dma(m4, split=False):
            for et in range(ET):
                t = qtp.tile([128, 512], F16, tag="qc", name=f"qc{m4}_{et}")
                if split:
                    nc.sync.dma_start(
                        t[:, 0:256], qT_d[bass.ts(et, 128), m4 * 512 : m4 * 512 + 256]
                    )
                else:
                    nc.sync.dma_start(t[:], qT_d[bass.ts(et, 128), bass.ts(m4, 512)])
                qc[m4][et] = t

        def vc_dma(q):
            for et in range(ET):
                t = vtp.tile([128, 512], F16, tag="vc", name=f"vc{q}_{et}")
                nc.gpsimd.dma_start(t[:], vT_d[bass.ts(et, 128), bass.ts(q, 512)])
                vc[q][et] = t

        def keep_dma(lo, hi):
            for nt in range(lo, hi):
                nc.sync.dma_start(
                    keep_sb[:, bass.ts(nt, M)], keepT_d[bass.ts(nt, 128), :]
                )

        kc_dma(0)
        qc_dma(0)
        keep_dma(0, 2)
        kc_dma(1)
        keep_dma(2, 4)
        kc_dma(2)
        keep_dma(4, 6)
        kc_dma(3)
        keep_dma(6, 9)
        qc_dma(1)
        keep_dma(9, 12)
        qc_dma(2)
        keep_dma(12, 16)
        qc_dma(3)
        vc_dma(0)
        vc_dma(1)
        vc_dma(2)
        vc_dma(3)

        # exp table warm-up (after DMA issue so it doesn't delay ScalarE's
        # weight dma_starts at t=0; the table load only must beat sweep 0)
        nc.vector.memset(warm[:], 1.0)
        nc.scalar.activation(warm[:], warm[:], EXP, scale=0.125)

        # ---- projection group emitters (dripped between score pairs) ----
        def k_group(d2, n4):
            ps = ps_pool.tile([128, 1024], F32, tag="sc", bufs=3, name=f"kp{d2}{n4}")
            for et in range(ET):
                nc.tensor.matmul(
                    ps[:, 0:512],
                    wk_sb[:, et * DLOC + d2 * 128 : et * DLOC + (d2 + 1) * 128],
                    kc[n4][et][:],
                    start=(et == 0), stop=(et == ET - 1),
                )
            nc.scalar.copy(kw_sb[d2][:, bass.ts(n4, 512)], ps[:, 0:512])

        def q_group(d2, m4):
            ps = ps_pool.tile([128, 1024], F32, tag="sc", bufs=3, name=f"qp{d2}{m4}")
            for et in range(ET):
                nc.tensor.matmul(
                    ps[:, 0:512],
                    wq_sb[:, et * DLOC + d2 * 128 : et * DLOC + (d2 + 1) * 128],
                    qc[m4][et][:],
                    start=(et == 0), stop=(et == ET - 1),
                )
            nc.vector.tensor_copy(qw_sb[d2][:, bass.ts(m4, 512)], ps[:, 0:512])

        def v_round(nt):
            q, r = divmod(nt, 4)
            ps = ps_pool.tile([128, 1024], F32, tag="sc", bufs=3, name=f"vp{nt}")
            for et in range(ET):
                nc.tensor.matmul(
                    ps[:, 0:DLOC],
                    vc[q][et][:, bass.ts(r, 128)],
                    wv_sb[:, bass.ts(et, DLOC)],
                    start=(et == 0), stop=(et == ET - 1),
                )
            src = ps[:, 0:DLOC].rearrange("p (h c) -> p h c", h=HL)
            dst = vw_sb[:, nt * VSTR : (nt + 1) * VSTR].rearrange(
                "p (h c) -> p h c", h=HL
            )[:, :, 0:DK]
            nc.scalar.copy(dst, src)

        def out_group(mt, split_copy=False):
            po = ps_pool.tile([128, 1024], F32, tag="sc", bufs=3, name=f"po{mt}")
            for kt2 in range(2):
                for ec in range(2):
                    nc.tensor.matmul(
                        po[:, bass.ts(ec, 512)],
                        ctx_sb[kt2][:, bass.ts(mt, 128)],
                        wo_sb[:, kt2 * E + ec * 512 : kt2 * E + (ec + 1) * 512],
                        start=(kt2 == 0), stop=(kt2 == 1),
                    )
            for ec in range(2):
                ob = ob_pool.tile([128, 512], F16, tag="ob", name=f"ob{mt}{ec}")
                if split_copy and ec == 1:
                    nc.scalar.copy(ob[:], po[:, bass.ts(ec, 512)])
                else:
                    nc.vector.tensor_copy(ob[:], po[:, bass.ts(ec, 512)])
                if split_copy:
                    nc.sync.dma_start(
                        out_d[bass.ts(mt, 128), ec * 512 : ec * 512 + 256],
                        ob[:, 0:256],
                    )
                    nc.sync.dma_start(
                        out_d[bass.ts(mt, 128), ec * 512 + 256 : (ec + 1) * 512],
                        ob[:, 256:512],
                    )
                else:
                    nc.sync.dma_start(
                        out_d[bass.ts(mt, 128), bass.ts(ec, 512)], ob[:]
                    )

        # drip schedule: sweep s = 2*mq + hp; drips[s][nt] = emitter
        def D(fn, *a):
            return lambda: fn(*a)

        drips = {s: {} for s in range(8)}
        drips[0] = {
            1: D(k_group, 0, 1), 2: D(k_group, 1, 0), 3: D(k_group, 0, 2),
            4: D(k_group, 1, 1), 5: D(k_group, 0, 3), 6: D(k_group, 1, 2),
            7: D(k_group, 1, 3), 9: D(q_group, 1, 0),
        }
        drips[1] = {3: D(q_group, 0, 1), 9: D(q_group, 1, 1)}
        drips[2] = {3: D(q_group, 0, 2), 9: D(q_group, 1, 2),
                    7: D(out_group, 0), 12: D(out_group, 1)}
        drips[3] = {3: D(q_group, 0, 3), 9: D(q_group, 1, 3),
                    7: D(out_group, 2), 12: D(out_group, 3)}
        drips[4] = {7: D(out_group, 4), 12: D(out_group, 5)}
        drips[5] = {7: D(out_group, 6), 12: D(out_group, 7)}
        drips[6] = {7: D(out_group, 8), 12: D(out_group, 9)}
        drips[7] = {7: D(out_group, 10), 12: D(out_group, 11)}

        # K(0,0) and Q(0,0) must precede sweep 0; V rounds drip per-nt inside.
        k_group(0, 0)
        q_group(0, 0)

        # ---- attention pair-sweeps ----
        # norm for sweep s runs piecewise inside sweep s+1 so the exp/mask
        # streams never queue behind the whole chain at a boundary.
        def norm_piece(state, piece):
            hp0, moff0, pctx0, csb0 = state
            if piece == 0:
                nc.vector.tensor_copy(csb0[0:65, :], pctx0[0:65, :])
            elif piece == 1:
                nc.vector.tensor_copy(sum_sb[:], csb0[64:65, :])
                nc.vector.reciprocal_approx_fast(sum_sb[:], sum_sb[:])
                nc.gpsimd.partition_broadcast(rbs_sb[:], sum_sb[0:1, :])
            else:
                i = piece - 2
                nc.vector.tensor_mul(
                    ctx_sb[hp0][bass.ts(i, 64), moff0 : moff0 + 512],
                    csb0[0:64, bass.ts(i, 512)],
                    rbs_sb[0:64, bass.ts(i, 512)],
                )

        pending = None
        for s in range(8):
            mq, hp = divmod(s, 2)
            moff = mq * 512
            pctx = ps_pool.tile([128, 1024], F32, tag="pctx", bufs=1, name=f"pctx{s}")
            ams = {}
            for step in range(NT + DEPTH):
                if step < NT and pending is not None and 1 <= step <= 4:
                    norm_piece(pending, step - 1)
                    if step == 4:
                        pending = None
                if step < NT:
                    nt = step
                    sc = ps_pool.tile(
                        [128, 1024], F32, tag="sc", bufs=3, name=f"sc{s}_{nt}"
                    )
                    # row-tiled concurrent pair: head A rows 0:64, B rows 64:128
                    nc.tensor.matmul(
                        sc[:, 0:512],
                        kw_sb[hp][0:64, bass.ts(nt, 128)],
                        qw_sb[hp][0:64, moff : moff + 512],
                        start=True, stop=True,
                    )
                    nc.tensor.matmul(
                        sc[:, 512:1024],
                        kw_sb[hp][64:128, bass.ts(nt, 128)],
                        qw_sb[hp][64:128, moff : moff + 512],
                        start=True, stop=True,
                    )
                    if s == 0 and nt >= 2:
                        v_round(nt - 2)
                    if s == 0 and nt == 15:
                        v_round(14)
                        v_round(15)
                    au = au_pool.tile([128, 1024], F16, tag="au", name=f"au{s}_{nt}")
                    nc.scalar.activation(au[:], sc[:], EXP, scale=0.125)
                    am = am_pool.tile([128, 1024], F16, tag="am", name=f"am{s}_{nt}")
                    ka, kb = bass.broadcast_tensor_aps(
                        au[:].rearrange("p (o c) -> p o c", o=2),
                        keep_sb[:, nt * M + moff : nt * M + moff + 512].rearrange(
                            "p (o c) -> p o c", o=1
                        ),
                    )
                    nc.vector.tensor_mul(
                        am[:].rearrange("p (o c) -> p o c", o=2), ka, kb
                    )
                    ams[nt] = am
                    fn = drips[s].get(nt)
                    if fn:
                        fn()
                if step >= DEPTH:
                    nt = step - DEPTH
                    am = ams.pop(nt)
                    for i in range(2):
                        h = 2 * hp + i
                        nc.tensor.matmul(
                            pctx[0:65, bass.ts(i, 512)],
                            vw_sb[:, nt * VSTR + h * 65 : nt * VSTR + (h + 1) * 65],
                            am[:, bass.ts(i, 512)],
                            start=(nt == 0), stop=(nt == NT - 1),
                        )
            # stash this sweep's normalize; it runs inside the next sweep
            csb = nrm_pool.tile([65, 1024], F32, tag="csb", name=f"csb{s}")
            pending = (hp, moff, pctx, csb)

        # final sweep's norm runs immediately, then the tail
        for piece in range(4):
            norm_piece(pending, piece)

        # ---- output projection tail (last m-chunk + leftovers) ----
        for mt in range(12, M // 128):
            out_group(mt, split_copy=True)

    nc.finalize()
    return nc


_PROGRAM = None


def _get_program():
    global _PROGRAM
    if _PROGRAM is None:
        _PROGRAM = build_program()
    return _PROGRAM


def _make_in_maps(q, k, v, mask, Wq, Wk, Wv, Wo):
    q = np.asarray(q, dtype=np.float32)
    k = np.asarray(k, dtype=np.float32)
    v = np.asarray(v, dtype=np.float32)
    mask = np.asarray(mask)
    Wq = np.asarray(Wq, dtype=np.float32)
    Wk = np.asarray(Wk, dtype=np.float32)
    Wv = np.asarray(Wv, dtype=np.float32)
    Wo = np.asarray(Wo, dtype=np.float32)

    per_batch = {}
    for b in range(B):
        per_batch[b] = dict(
            qT=np.ascontiguousarray(q[b].T.astype(np.float16)),
            kT=np.ascontiguousarray(k[b].T.astype(np.float16)),
            vT=np.ascontiguousarray(v[b].T.astype(np.float16)),
            keepT=np.ascontiguousarray(
                np.logical_not(mask[b]).T.astype(np.float16)
            ),
        )

    in_maps = []
    for c in range(NCORES):
        b, hg = divmod(c, GROUPS)
        sl = slice(hg * DLOC, (hg + 1) * DLOC)
        in_maps.append(
            dict(
                per_batch[b],
                wqT=np.ascontiguousarray(Wq[sl].T.astype(np.float16)),
                wkT=np.ascontiguousarray(Wk[sl].T.astype(np.float16)),
                wvT=np.ascontiguousarray(Wv[sl].T.astype(np.float16)),
                woT=np.ascontiguousarray(Wo[:, sl].T.astype(np.float16)),
            )
        )
    return in_maps


def _run(in_maps, trace=False):
    nc = _get_program()
    return run_bass_kernel_spmd(
        nc, in_maps, list(range(NCORES)), trace=trace
    )


def _assemble(results):
    out = np.zeros((B, M, E), dtype=np.float32)
    for c in range(NCORES):
        b = c // GROUPS
        out[b] += results[c]["out"].astype(np.float32)
    return out


def kernel(q, k, v, mask, Wq, Wk, Wv, Wo):
    in_maps = _make_in_maps(q, k, v, mask, Wq, Wk, Wv, Wo)
    res = _run(in_maps, trace=False)
    return _assemble(res.results)


def run_profiled(q, k, v, mask, Wq, Wk, Wv, Wo):
    """Like kernel(), but traces execution; returns (out, BassKernelResults)."""
    in_maps = _make_in_maps(q, k, v, mask, Wq, Wk, Wv, Wo)
    res = _run(in_maps, trace=True)
    return _assemble(res.results), res


# revision 45
# speedup vs baseline: 1.0546x; 1.0216x over previous
"""Multi-head attention TRN2 Bass kernel (8 NeuronCores, tensor-parallel).

Sharding: Megatron-style TP over (batch x head-group). 8 cores = 2 batches x 4
head-groups of 4 heads each. Each core computes its heads' Q/K/V projections,
masked-softmax attention, and a partial output projection; the host sums the 4
partials per batch (the TP unshard).

Final layout (pair-sweeps + row-tiled scores + JIT streaming):
  - Attention runs as 8 sweeps over (mq 512-wide m-chunk, hp head-pair). Per
    n-tile, the pair's two score matmuls are K=64 row-tiled (PE row groups 0
    and 64) and run CONCURRENTLY into the two banks of one sc tile; exp and
    the mask multiply treat the [A|B] tile as one 1024-wide op; ctx for the
    two heads lands in the column halves of one pctx tile (ones-row carries
    softmax sums).
  - Inputs stream column-major just-in-time: K-projection groups and
    V-projection rounds drip into sweep 0 between score pairs; Q-projection
    groups and the output projection drip into later sweeps.
  - DMA issue is spread across SP (kt/qt/out), ScalarE (wk/wq, ramp only)
    and GpSimd SWDGE (wv/wo/vt/keep) to beat the per-engine ~0.6us/issue
    serialization.
  - Normalize: one csb evacuation frees pctx fast; reciprocal_approx_fast on
    the sums row (partition 0), gpsimd partition_broadcast, two fused
    evacuate-multiplies. PSUM = sc[128,1024]f32 x3 + pctx x1 = 8 banks.
"""
import sys

for p in ("/opt/trn_rl_repo",):
    if p not in sys.path:
        sys.path.insert(0, p)

from contextlib import ExitStack

import numpy as np

import concourse.bass as bass
import concourse.tile as tile
from concourse import bacc, mybir
from concourse.bass_utils import run_bass_kernel_spmd

F32 = mybir.dt.float32
F16 = mybir.dt.float16
EXP = mybir.ActivationFunctionType.Exp

B, M, N, E = 2, 2048, 2048, 1024  # batch, q-len, k-len, d_model
H, DK = 16, 64                    # heads, head dim
NCORES = 8
GROUPS = 4                        # head groups (cores per batch)
DLOC = (H // GROUPS) * DK         # 256 per-core projection width
HL = H // GROUPS                  # 4 local heads
ET = E // 128                     # 8 e-tiles
NT = N // 128                     # 16 n-tiles
VSTR = HL * (DK + 1)              # 260: vw slot stride per n-tile
DEPTH = 4                         # ctx lag behind scores in the nt pipeline


def build_program() -> bass.Bass:
    nc = bacc.Bacc()

    qT_d = nc.dram_tensor("qT", [E, M], F16, kind="ExternalInput")
    kT_d = nc.dram_tensor("kT", [E, N], F16, kind="ExternalInput")
    vT_d = nc.dram_tensor("vT", [E, N], F16, kind="ExternalInput")
    keepT_d = nc.dram_tensor("keepT", [N, M], F16, kind="ExternalInput")
    wqT_d = nc.dram_tensor("wqT", [E, DLOC], F16, kind="ExternalInput")
    wkT_d = nc.dram_tensor("wkT", [E, DLOC], F16, kind="ExternalInput")
    wvT_d = nc.dram_tensor("wvT", [E, DLOC], F16, kind="ExternalInput")
    woT_d = nc.dram_tensor("woT", [DLOC, E], F16, kind="ExternalInput")
    out_d = nc.dram_tensor("out", [M, E], F16, kind="ExternalOutput")

    with tile.TileContext(nc) as tc, ExitStack() as ctx:
        persist = ctx.enter_context(tc.tile_pool(name="persist", bufs=1))
        ktp = ctx.enter_context(tc.tile_pool(name="ktp", bufs=24))
        vtp = ctx.enter_context(tc.tile_pool(name="vtp", bufs=16))
        qtp = ctx.enter_context(tc.tile_pool(name="qtp", bufs=16))
        ps_pool = ctx.enter_context(tc.tile_pool(name="ps", bufs=1, space="PSUM"))
        nrm_pool = ctx.enter_context(tc.tile_pool(name="nrm", bufs=1))
        au_pool = ctx.enter_context(tc.tile_pool(name="au", bufs=4))
        am_pool = ctx.enter_context(tc.tile_pool(name="am", bufs=5))
        ob_pool = ctx.enter_context(tc.tile_pool(name="ob", bufs=4))

        warm = persist.tile([1, 64], F16, tag="warm")
        wq_sb = persist.tile([128, ET * DLOC], F16, tag="wq")
        wk_sb = persist.tile([128, ET * DLOC], F16, tag="wk")
        wv_sb = persist.tile([128, ET * DLOC], F16, tag="wv")
        wo_sb = persist.tile([128, 2 * E], F16, tag="wo")

        # qw/kw: [d-rows, seq]; pair hp = heads {2hp, 2hp+1} = top/bottom
        # 64-partition halves — exactly the row-tiling split.
        qw_sb = [persist.tile([128, M], F16, tag=f"qw{i}", name=f"qw{i}") for i in range(2)]
        kw_sb = [persist.tile([128, N], F16, tag=f"kw{i}", name=f"kw{i}") for i in range(2)]
        vw_sb = persist.tile([128, NT * VSTR], F16, tag="vw")
        nc.vector.memset(vw_sb[:], 1.0)  # ones cols persist; data overwritten
        ctx_sb = [persist.tile([128, M], F16, tag=f"ctx{i}", name=f"ctx{i}") for i in range(2)]
        keep_sb = persist.tile([128, NT * M], F16, tag="keep")
        sum_sb = persist.tile([1, 1024], F32, tag="sum")
        rbs_sb = persist.tile([64, 1024], F32, tag="rbs")

        # ---- DMA issue: ACT: wk+wq; GpSimd: wv+wo+vt+keep; SP: kt+qt ----
        for et in range(ET):
            nc.scalar.dma_start(wk_sb[:, bass.ts(et, DLOC)], wkT_d[bass.ts(et, 128), :])
        for et in range(ET):
            nc.scalar.dma_start(wq_sb[:, bass.ts(et, DLOC)], wqT_d[bass.ts(et, 128), :])
        for et in range(ET):
            nc.gpsimd.dma_start(wv_sb[:, bass.ts(et, DLOC)], wvT_d[bass.ts(et, 128), :])
        for kt2 in range(2):
            nc.gpsimd.dma_start(wo_sb[:, bass.ts(kt2, E)], woT_d[bass.ts(kt2, 128), :])

        # column chunks: kc[n4][et], qc[m4][et] on SP; vc[q][et] + keep on
        # gpsimd. Issue order is need order: kc0+qc0 first, keep interleaved
        # with vc so the first masks aren't starved behind 32 vc issues.
        kc = [[None] * ET for _ in range(4)]
        qc = [[None] * ET for _ in range(4)]
        vc = [[None] * ET for _ in range(4)]

        def kc_dma(n4, split=False):
            for et in range(ET):
                t = ktp.tile([128, 512], F16, tag="kc", name=f"kc{n4}_{et}")
                if split:
                    nc.sync.dma_start(
                        t[:, 0:256], kT_d[bass.ts(et, 128), n4 * 512 : n4 * 512 + 256]
                    )
                else:
                    nc.sync.dma_start(t[:], kT_d[bass.ts(et, 128), bass.ts(n4, 512)])
                kc[n4][et] = t

        def qc_dma(m4, split=False):
            for et in range(ET):
                t = qtp.tile([128, 512], F16, tag="qc", name=f"qc{m4}_{et}")
                if split:
                    nc.sync.dma_start(
                        t[:, 0:256], qT_d[bass.ts(et, 128), m4 * 512 : m4 * 512 + 256]
                    )
                else:
                    nc.sync.dma_start(t[:], qT_d[bass.ts(et, 128), bass.ts(m4, 512)])
                qc[m4][et] = t

        def vc_dma(q):
            for et in range(ET):
                t = vtp.tile([128, 512], F16, tag="vc", name=f"vc{q}_{et}")
                nc.gpsimd.dma_start(t[:], vT_d[bass.ts(et, 128), bass.ts(q, 512)])
                vc[q][et] = t

        def keep_dma(lo, hi):
            for nt in range(lo, hi):
                nc.sync.dma_start(
                    keep_sb[:, bass.ts(nt, M)], keepT_d[bass.ts(nt, 128), :]
                )

        kc_dma(0)
        qc_dma(0)
        keep_dma(0, 2)
        kc_dma(1)
        keep_dma(2, 4)
        kc_dma(2)
        keep_dma(4, 6)
        kc_dma(3)
        keep_dma(6, 9)
        qc_dma(1)
        keep_dma(9, 12)
        qc_dma(2)
        keep_dma(12, 16)
        qc_dma(3)
        vc_dma(0)
        vc_dma(1)
        vc_dma(2)
        vc_dma(3)

        # exp table warm-up (after DMA issue so it doesn't delay ScalarE's
        # weight dma_starts at t=0; the table load only must beat sweep 0)
        nc.vector.memset(warm[:], 1.0)
        nc.scalar.activation(warm[:], warm[:], EXP, scale=0.125)

        # ---- projection group emitters (dripped between score pairs) ----
        def k_group(d2, n4):
            ps = ps_pool.tile([128, 1024], F32, tag="sc", bufs=3, name=f"kp{d2}{n4}")
            for et in range(ET):
                nc.tensor.matmul(
                    ps[:, 0:512],
                    wk_sb[:, et * DLOC + d2 * 128 : et * DLOC + (d2 + 1) * 128],
                    kc[n4][et][:],
                    start=(et == 0), stop=(et == ET - 1),
                )
            nc.scalar.copy(kw_sb[d2][:, bass.ts(n4, 512)], ps[:, 0:512])

        def q_group(d2, m4):
            ps = ps_pool.tile([128, 1024], F32, tag="sc", bufs=3, name=f"qp{d2}{m4}")
            for et in range(ET):
                nc.tensor.matmul(
                    ps[:, 0:512],
                    wq_sb[:, et * DLOC + d2 * 128 : et * DLOC + (d2 + 1) * 128],
                    qc[m4][et][:],
                    start=(et == 0), stop=(et == ET - 1),
                )
            nc.vector.tensor_copy(qw_sb[d2][:, bass.ts(m4, 512)], ps[:, 0:512])

        def v_round(nt):
            q, r = divmod(nt, 4)
            ps = ps_pool.tile([128, 1024], F32, tag="sc", bufs=3, name=f"vp{nt}")
            for et in range(ET):
                nc.tensor.matmul(
                    ps[:, 0:DLOC],
                    vc[q][et][:, bass.ts(r, 128)],
                    wv_sb[:, bass.ts(et, DLOC)],
                    start=(et == 0), stop=(et == ET - 1),
                )
            src = ps[:, 0:DLOC].rearrange("p (h c) -> p h c", h=HL)
            dst = vw_sb[:, nt * VSTR : (nt + 1) * VSTR].rearrange(
                "p (h c) -> p h c", h=HL
            )[:, :, 0:DK]
            nc.scalar.copy(dst, src)

        def out_group(mt, split_copy=False):
            po = ps_pool.tile([128, 1024], F32, tag="sc", bufs=3, name=f"po{mt}")
            for kt2 in range(2):
                for ec in range(2):
                    nc.tensor.matmul(
                        po[:, bass.ts(ec, 512)],
                        ctx_sb[kt2][:, bass.ts(mt, 128)],
                        wo_sb[:, kt2 * E + ec * 512 : kt2 * E + (ec + 1) * 512],
                        start=(kt2 == 0), stop=(kt2 == 1),
                    )
            for ec in range(2):
                ob = ob_pool.tile([128, 512], F16, tag="ob", name=f"ob{mt}{ec}")
                if split_copy and ec == 1:
                    nc.scalar.copy(ob[:], po[:, bass.ts(ec, 512)])
                else:
                    nc.vector.tensor_copy(ob[:], po[:, bass.ts(ec, 512)])
                if split_copy:
                    nc.sync.dma_start(
                        out_d[bass.ts(mt, 128), ec * 512 : ec * 512 + 256],
                        ob[:, 0:256],
                    )
                    nc.sync.dma_start(
                        out_d[bass.ts(mt, 128), ec * 512 + 256 : (ec + 1) * 512],
                        ob[:, 256:512],
                    )
                else:
                    nc.sync.dma_start(
                        out_d[bass.ts(mt, 128), bass.ts(ec, 512)], ob[:]
                    )

        # drip schedule: sweep s = 2*mq + hp; drips[s][nt] = emitter
        def D(fn, *a):
            return lambda: fn(*a)

        drips = {s: {} for s in range(8)}
        drips[0] = {
            1: D(k_group, 0, 1), 2: D(k_group, 1, 0), 3: D(k_group, 0, 2),
            4: D(k_group, 1, 1), 5: D(k_group, 0, 3), 6: D(k_group, 1, 2),
            7: D(k_group, 1, 3), 9: D(q_group, 1, 0),
        }
        drips[1] = {3: D(q_group, 0, 1), 9: D(q_group, 1, 1)}
        drips[2] = {3: D(q_group, 0, 2), 9: D(q_group, 1, 2),
                    7: D(out_group, 0), 12: D(out_group, 1)}
        drips[3] = {3: D(q_group, 0, 3), 9: D(q_group, 1, 3),
                    7: D(out_group, 2), 12: D(out_group, 3)}
        drips[4] = {7: D(out_group, 4), 12: D(out_group, 5)}
        drips[5] = {7: D(out_group, 6), 12: D(out_group, 7)}
        drips[6] = {7: D(out_group, 8), 12: D(out_group, 9)}
        drips[7] = {7: D(out_group, 10), 12: D(out_group, 11)}

        # K(0,0) and Q(0,0) must precede sweep 0; V rounds drip per-nt inside.
        k_group(0, 0)
        q_group(0, 0)

        # ---- attention pair-sweeps ----
        # norm for sweep s runs piecewise inside sweep s+1 so the exp/mask
        # streams never queue behind the whole chain at a boundary.
        def norm_piece(state, piece):
            hp0, moff0, pctx0, csb0 = state
            if piece == 0:
                nc.vector.tensor_copy(csb0[0:65, :], pctx0[0:65, :])
            elif piece == 1:
                nc.vector.tensor_copy(sum_sb[:], csb0[64:65, :])
                nc.vector.reciprocal_approx_fast(sum_sb[:], sum_sb[:])
                nc.gpsimd.partition_broadcast(rbs_sb[:], sum_sb[0:1, :])
            else:
                i = piece - 2
                nc.vector.tensor_mul(
                    ctx_sb[hp0][bass.ts(i, 64), moff0 : moff0 + 512],
                    csb0[0:64, bass.ts(i, 512)],
                    rbs_sb[0:64, bass.ts(i, 512)],
                )

        pending = None
        for s in range(8):
            mq, hp = divmod(s, 2)
            moff = mq * 512
            pctx = ps_pool.tile([128, 1024], F32, tag="pctx", bufs=1, name=f"pctx{s}")
            ams = {}
            for step in range(NT + DEPTH):
                if step < NT and pending is not None and 1 <= step <= 4:
                    norm_piece(pending, step - 1)
                    if step == 4:
                        pending = None
                if step < NT:
                    nt = step
                    sc = ps_pool.tile(
                        [128, 1024], F32, tag="sc", bufs=3, name=f"sc{s}_{nt}"
                    )
                    # row-tiled concurrent pair: head A rows 0:64, B rows 64:128
                    nc.tensor.matmul(
                        sc[:, 0:512],
                        kw_sb[hp][0:64, bass.ts(nt, 128)],
                        qw_sb[hp][0:64, moff : moff + 512],
                        start=True, stop=True,
                    )
                    nc.tensor.matmul(
                        sc[:, 512:1024],
                        kw_sb[hp][64:128, bass.ts(nt, 128)],
                        qw_sb[hp][64:128, moff : moff + 512],
                        start=True, stop=True,
                    )
                    if s == 0 and nt >= 2:
                        v_round(nt - 2)
                    if s == 0 and nt == 15:
                        v_round(14)
                        v_round(15)
                    au = au_pool.tile([128, 1024], F16, tag="au", name=f"au{s}_{nt}")
                    nc.scalar.activation(au[:], sc[:], EXP, scale=0.125)
                    am = am_pool.tile([128, 1024], F16, tag="am", name=f"am{s}_{nt}")
                    ka, kb = bass.broadcast_tensor_aps(
                        au[:].rearrange("p (o c) -> p o c", o=2),
                        keep_sb[:, nt * M + moff : nt * M + moff + 512].rearrange(
                            "p (o c) -> p o c", o=1
                        ),
                    )
                    nc.vector.tensor_mul(
                        am[:].rearrange("p (o c) -> p o c", o=2), ka, kb
                    )
                    ams[nt] = am
                    fn = drips[s].get(nt)
                    if fn:
                        fn()
                if step >= DEPTH:
                    nt = step - DEPTH
                    am = ams.pop(nt)
                    for i in range(2):
                        h = 2 * hp + i
                        nc.tensor.matmul(
                            pctx[0:65, bass.ts(i, 512)],
                            vw_sb[:, nt * VSTR + h * 65 : nt * VSTR + (h + 1) * 65],
                            am[:, bass.ts(i, 512)],
                            start=(nt == 0), stop=(nt == NT - 1),
                        )
            # stash this sweep's normalize; it runs inside the next sweep
            csb = nrm_pool.tile([65, 1024], F32, tag="csb", name=f"csb{s}")
            pending = (hp, moff, pctx, csb)

        # final sweep's norm runs immediately, then the tail
        for piece in range(4):
            norm_piece(pending, piece)

        # ---- output projection tail (last m-chunk + leftovers) ----
        for mt in range(12, M // 128):
            out_group(mt, split_copy=True)

    nc.finalize()
    return nc


_PROGRAM = None


def _get_program():
    global _PROGRAM
    if _PROGRAM is None:
        _PROGRAM = build_program()
    return _PROGRAM


def _make_in_maps(q, k, v, mask, Wq, Wk, Wv, Wo):
    q = np.asarray(q, dtype=np.float32)
    k = np.asarray(k, dtype=np.float32)
    v = np.asarray(v, dtype=np.float32)
    mask = np.asarray(mask)
    Wq = np.asarray(Wq, dtype=np.float32)
    Wk = np.asarray(Wk, dtype=np.float32)
    Wv = np.asarray(Wv, dtype=np.float32)
    Wo = np.asarray(Wo, dtype=np.float32)

    per_batch = {}
    for b in range(B):
        per_batch[b] = dict(
            qT=np.ascontiguousarray(q[b].T.astype(np.float16)),
            kT=np.ascontiguousarray(k[b].T.astype(np.float16)),
            vT=np.ascontiguousarray(v[b].T.astype(np.float16)),
            keepT=np.ascontiguousarray(
                np.logical_not(mask[b]).T.astype(np.float16)
            ),
        )

    in_maps = []
    for c in range(NCORES):
        b, hg = divmod(c, GROUPS)
        sl = slice(hg * DLOC, (hg + 1) * DLOC)
        in_maps.append(
            dict(
                per_batch[b],
                wqT=np.ascontiguousarray(Wq[sl].T.astype(np.float16)),
                wkT=np.ascontiguousarray(Wk[sl].T.astype(np.float16)),
                wvT=np.ascontiguousarray(Wv[sl].T.astype(np.float16)),
                woT=np.ascontiguousarray(Wo[:, sl].T.astype(np.float16)),
            )
        )
    return in_maps


def _run(in_maps, trace=False):
    nc = _get_program()
    return run_bass_kernel_spmd(
        nc, in_maps, list(range(NCORES)), trace=trace
    )


def _assemble(results):
    out = np.zeros((B, M, E), dtype=np.float32)
    for c in range(NCORES):
        b = c // GROUPS
        out[b] += results[c]["out"].astype(np.float32)
    return out


def kernel(q, k, v, mask, Wq, Wk, Wv, Wo):
    in_maps = _make_in_maps(q, k, v, mask, Wq, Wk, Wv, Wo)
    res = _run(in_maps, trace=False)
    return _assemble(res.results)


def run_profiled(q, k, v, mask, Wq, Wk, Wv, Wo):
    """Like kernel(), but traces execution; returns (out, BassKernelResults)."""
    in_maps = _make_in_maps(q, k, v, mask, Wq, Wk, Wv, Wo)
    res = _run(in_maps, trace=True)
    return _assemble(res.results), res
